# revision 1
# baseline (speedup 1.0000x reference)
"""DiT forward kernel for Trainium2, data-parallel over batch (1 sample/core x 8 cores).

- Each core runs the full transformer for one batch element; weights replicated.
- Activations feature-major [C, N=256]; weights host-pretransposed to [Cin, Cout].
- Matmuls in float32r (full PE rate at N>=256, ~1e-4 rel err vs fp32).
- Partition reductions (rmsnorm/layernorm/softmax-sum) via ones-matmuls on PE;
  partition broadcasts via K=1/selector matmuls; rotate_half via a constant
  permutation matmul with qn/kn weights and signs folded in.
- Weights streamed as column chunks [1024, 512] so tile pools cycle cleanly.
"""
import sys
sys.path.insert(0, '/opt/trn_rl_repo')
import numpy as np
import concourse.bacc as bacc
import concourse.mybir as mybir
from concourse.tile import TileContext
from concourse import bass_utils

F32 = mybir.dt.float32
F32R = mybir.dt.float32r
AF = mybir.ActivationFunctionType
OP = mybir.AluOpType

P, NH, HD, OC, D = 2, 16, 64, 4, 4
B, C, N, SW = 8, 1024, 256, 2730
EPS_RMS, EPS_LN = 1e-6, 1e-5
KT = C // 128            # 8 cin tiles
SWT = (SW + 127) // 128  # 22 sw tiles (last has 42 rows)
USE_F32R = True


def build_nc(debug_stage=None):
    nc = bacc.Bacc(None, target_bir_lowering=False)
    dt = {}

    def din(name, shape):
        dt[name] = nc.dram_tensor(name, list(shape), F32, kind="ExternalInput")
        return dt[name]

    din("patches", (16, N)); din("pos2", (C, N)); din("siluc", (128, KT))
    din("wqkvT", (D, C, 3 * C)); din("wprojT", (D, C, C)); din("projbr", (D, 1, C))
    din("w1T", (D, C, SW)); din("w3T", (D, C, SW)); din("w2T", (D, SW, C))
    din("adaT", (D, C, 6 * C)); din("adabc", (D, 128, 48))
    din("n1w", (D, 128, KT)); din("n2w", (D, 128, KT))
    din("coswq", (D, 128, N)); din("coswk", (D, 128, N)); din("sinm", (128, N))
    din("protq", (D, 128, 128)); din("protk", (D, 128, 128))
    din("hsum", (128, 2)); din("sel2", (2, 128)); din("halvesrow", (1, 2 * 128))
    din("onesrow", (1, N)); din("onescol", (128, 1)); din("vones", (128, 32)); din("ident", (128, 128)); din("epsc", (128, 2))
    din("patchWT", (16, C))
    din("finadaT", (C, 2 * C)); din("finadabc", (128, 16))
    din("finT", (C, 16)); din("finb", (16, 1))
    out_d = nc.dram_tensor("out", [16, N], F32, kind="ExternalOutput")
    dbg_d = nc.dram_tensor("dbg", [128, KT, N], F32, kind="ExternalOutput") if debug_stage else None

    def dump(stage, ap):
        if debug_stage == stage:
            nc.sync.dma_start(dbg_d[:] if list(ap.shape) == [128, KT, N] else dbg_d[:, 0, 0:list(ap.shape)[-1]], ap)

    MMDT = F32R if USE_F32R else F32

    def mm(out, lhsT, rhs, **kw):
        nc.tensor.matmul(out, lhsT, rhs, **kw)

    ctx_lp = nc.allow_low_precision("fp32r matmul inputs")
    ctx_lp.__enter__()
    with TileContext(nc) as tc:
        with (
            tc.sbuf_pool(name="cst", bufs=1) as cst,
            tc.sbuf_pool(name="wch", bufs=3) as wch,   # [128, KT, 512] column chunks
            tc.sbuf_pool(name="w2c", bufs=2) as w2c,   # [128, 22, 256] w2 chunks
            tc.sbuf_pool(name="act", bufs=1) as act,
            tc.sbuf_pool(name="tp", bufs=2) as tp,
            tc.tile_pool(name="drm", bufs=1, space="DRAM") as drm,
            tc.psum_pool(name="pmm", bufs=4) as pmm,
            tc.psum_pool(name="pat", bufs=2) as pat,
            tc.psum_pool(name="pbc", bufs=2) as pbc,
        ):
            # ---- constants ----
            def cload(name, shape, dtype=F32, **kw):
                t = cst.tile(shape, dtype, name=f"c_{name}", tag=f"c_{name}", **kw)
                src_ap = dt[name][:]
                if dtype is not F32:
                    src_ap = src_ap.bitcast(dtype)
                nc.sync.dma_start(t, src_ap)
                return t
            patches = cload("patches", [16, N], dtype=MMDT)
            pos2 = cst.tile([128, KT, N], F32, name="c_pos2", tag="c_pos2")
            nc.sync.dma_start(pos2, dt["pos2"][:].rearrange("(a p) n -> p a n", p=128))
            siluc = cload("siluc", [128, KT], dtype=MMDT)
            hsum = cload("hsum", [128, 2], dtype=MMDT)
            sel2 = cload("sel2", [2, 128], dtype=MMDT)
            halvesrow = cload("halvesrow", [1, 2 * 128], dtype=MMDT)
            onesrow = cload("onesrow", [1, N], dtype=MMDT)
            onescol = cload("onescol", [128, 1], dtype=MMDT)
            ident = cload("ident", [128, 128], dtype=MMDT)
            epsc = cload("epsc", [128, 2])
            patchWT = cload("patchWT", [16, C], dtype=MMDT)
            sinm = cload("sinm", [128, N])
            finT = cst.tile([128, KT, 16], MMDT, name="c_finT", tag="c_finT")
            nc.sync.dma_start(finT, dt["finT"][:].rearrange("(a p) n -> p a n", p=128).bitcast(MMDT))
            finb = cload("finb", [16, 1])
            finadabc = cload("finadabc", [128, 16])

            # ---- long-lived activations ----
            xc = act.tile([128, KT, N], MMDT, name="xc")
            xm = act.tile([128, KT, N], MMDT, name="xm")
            qk = act.tile([128, 2 * KT, N], MMDT, name="qk")
            vaug = act.tile([128, 2, NH * (HD + 1)], MMDT, name="vaug")
            attno = act.tile([128, KT, N], MMDT, name="attno")
            h1 = act.tile([128, SWT, N], MMDT, name="h1")
            xsq = h1[:, KT:2 * KT, :]  # alias: xsq uses precede h1 writes each block
            modscol = act.tile([128, 48], F32, name="modscol")
            rskT = act.tile([128, 2, 16], F32, name="rskT")
            a1 = act.tile([128, KT], F32, name="a1")
            afin = act.tile([128, KT], F32, name="afin")
            fincol = act.tile([128, 16], F32, name="fincol")
            modsdram = drm.tile([1, 6 * C], F32, name="modsdram")

            nc.sync.dma_start(
                vaug.rearrange("p t (h d) -> p t h d", d=HD + 1)[:, :, :, HD:HD + 1],
                dt["vones"][:].bitcast(MMDT).rearrange("p (t h o) -> p t h o", t=2, h=NH))

            # ================= patch embed =================
            for co in range(KT):
                acc = pmm.tile([128, N], F32, tag="mm", name="acc_emb")
                mm(acc, patchWT[:, co * 128:(co + 1) * 128], patches, start=True, stop=True)
                nc.vector.tensor_tensor(xc[:, co, :], acc, pos2[:, co, :], op=OP.add)

            dump("xc0", xc.bitcast(F32))

            def wchunk(dram_ap, cols, c0, cw, name):
                """Load [C, cw] column chunk as [128, KT, cw] (tag-shared [*,*,512] slot)."""
                t = wch.tile([128, KT, cw], MMDT, tag="wch", name=name,
                             padded_shape=[128, KT, 512])
                nc.sync.dma_start(t, dram_ap.rearrange("(a p) n -> p a n", p=128)[:, :, c0:c0 + cw].bitcast(MMDT))
                return t

            def ln_modulate(dst, rms_only, nw_dram, acol, mc, s_shift, s_scale):
                """dst = norm(xc)*(w*(1+scale_mod)) + shift_mod (feature-major)."""
                for i in range(KT):
                    nc.gpsimd.tensor_tensor(xsq[:, i, :], xc[:, i, :], xc[:, i, :], op=OP.mult)
                ssum = pbc.tile([1, N], F32, tag="bc", name="ssum")
                for i in range(KT):
                    mm(ssum, onescol, xsq[:, i, :], start=(i == 0), stop=(i == KT - 1))
                if rms_only:
                    sq = tp.tile([1, N], F32, tag="sq", name="sq", bufs=1)
                    nc.scalar.activation(sq, ssum, AF.Sqrt, scale=1.0 / C, bias=epsc[0:1, 0:1])
                else:
                    s1 = pbc.tile([1, N], F32, tag="bc", name="s1")
                    for i in range(KT):
                        mm(s1, onescol, xc[:, i, :], start=(i == 0), stop=(i == KT - 1))
                    mrow = tp.tile([1, N], F32, tag="mrow", name="mrow", bufs=1)
                    nc.scalar.activation(mrow, s1, AF.Copy, scale=1.0 / C)
                    m2 = tp.tile([1, N], F32, tag="m2", name="m2", bufs=1)
                    nc.scalar.activation(m2, mrow, AF.Square)
                    vrow = tp.tile([1, N], F32, tag="vrow", name="vrow", bufs=1)
                    nc.vector.scalar_tensor_tensor(vrow, ssum, 1.0 / C, m2,
                                                   op0=OP.mult, op1=OP.subtract)
                    sq = tp.tile([1, N], F32, tag="sq", name="sq", bufs=1)
                    nc.scalar.activation(sq, vrow, AF.Sqrt, bias=epsc[0:1, 1:2])
                rsrow = tp.tile([1, N], MMDT, tag="rsrow", name="rsrow", bufs=1)
                nc.vector.reciprocal(rsrow, sq)
                rbc = pbc.tile([128, N], F32, tag="bc", name="rbc")
                mm(rbc, onesrow[:, 0:128], rsrow, start=True, stop=True)
                if not rms_only:
                    mrs = tp.tile([1, N], MMDT, tag="mrs", name="mrs", bufs=1)
                    nc.vector.tensor_tensor(mrs, mrow, rsrow, op=OP.mult)
                    mbc = pbc.tile([128, N], F32, tag="bc", name="mbc")
                    mm(mbc, onesrow[:, 0:128], mrs, start=True, stop=True)
                if nw_dram is None:
                    nc.vector.tensor_scalar_add(acol, mc[:, s_scale:s_scale + KT], 1.0)
                else:
                    aw = cst.tile([128, KT], F32, tag="aw", name="aw", bufs=4)
                    nc.sync.dma_start(aw, nw_dram)
                    nc.vector.scalar_tensor_tensor(acol, mc[:, s_scale:s_scale + KT], 1.0, aw,
                                                   op0=OP.add, op1=OP.mult)
                for i in range(KT):
                    tmp = tp.tile([128, N], F32, tag="lnt", name="lnt")
                    nc.vector.tensor_tensor(tmp, xc[:, i, :], rbc, op=OP.mult)
                    if not rms_only:
                        nc.vector.tensor_tensor(tmp, tmp, mbc, op=OP.subtract)
                    nc.scalar.activation(dst[:, i, :], tmp, AF.Identity,
                                         scale=acol[:, i:i + 1],
                                         bias=mc[:, s_shift * KT + i:s_shift * KT + i + 1])

            def ada_mods(dram_w, ncols, dst_dram):
                nch = ncols // 512
                for ch in range(nch):
                    pan = wchunk(dram_w, ncols, ch * 512, 512, "adach")
                    macc = pbc.tile([1, 512], F32, tag="bc", name="macc")
                    for k in range(KT):
                        mm(macc, siluc[:, k:k + 1], pan[:, k, :], start=(k == 0), stop=(k == KT - 1))
                    stg = tp.tile([1, 512], F32, tag="mstg", name="mstg", bufs=1)
                    nc.vector.tensor_copy(stg, macc)
                    nc.sync.dma_start(dst_dram[0:1, ch * 512:(ch + 1) * 512], stg)

            # ================= transformer blocks =================
            for b in range(D):
                # ---- ada mods ----
                ada_mods(dt["adaT"][b], 6 * C, modsdram)
                nc.sync.dma_start(modscol,
                                  modsdram[0:1, :].rearrange("o (j p) -> o p j", p=128)[0])
                adab = cst.tile([128, 48], F32, tag="adab", name="adab", bufs=2)
                nc.sync.dma_start(adab, dt["adabc"][b])
                nc.vector.tensor_tensor(modscol, modscol, adab, op=OP.add)
                if b == 0:
                    dump("mods0", modscol)

                # ---- attn branch ----
                ln_modulate(xm, True, dt["n1w"][b], a1, modscol, 0, 8)

                for ch in range(6):  # qkv column chunks of 512
                    pan = wchunk(dt["wqkvT"][b], 3 * C, ch * 512, 512, "qkvch")
                    if ch < 4:  # q, k feature-major
                        for j in range(4):
                            co = ch * 4 + j
                            acc = pmm.tile([128, N], F32, tag="mm", name="acc_qk")
                            for k in range(KT):
                                mm(acc, pan[:, k, j * 128:(j + 1) * 128], xm[:, k, :],
                                   start=(k == 0), stop=(k == KT - 1))
                            nc.any.tensor_copy(qk[:, co, :], acc)
                    else:  # v token-major into vaug
                        for tq in range(2):
                            acc = pmm.tile([128, 512], F32, tag="mm", name="acc_v")
                            for k in range(KT):
                                mm(acc, xm[:, k, tq * 128:(tq + 1) * 128], pan[:, k, :],
                                   start=(k == 0), stop=(k == KT - 1))
                            h0 = (ch - 4) * 8
                            nc.any.tensor_copy(
                                vaug[:, tq, :].rearrange("p (h d) -> p h d", d=HD + 1)
                                [:, h0:h0 + 8, 0:HD],
                                acc.rearrange("p (h d) -> p h d", d=HD))

                # ---- q/k rmsnorm + rope ----
                coswq = cst.tile([128, N], F32, tag="coswq", name="coswq", bufs=2)
                nc.sync.dma_start(coswq, dt["coswq"][b])
                coswk = cst.tile([128, N], F32, tag="coswk", name="coswk", bufs=2)
                nc.sync.dma_start(coswk, dt["coswk"][b])
                protq = cst.tile([128, 128], MMDT, tag="protq", name="protq", bufs=2)
                nc.sync.dma_start(protq, dt["protq"][b].bitcast(MMDT))
                protk = cst.tile([128, 128], MMDT, tag="protk", name="protk", bufs=2)
                nc.sync.dma_start(protk, dt["protk"][b].bitcast(MMDT))

                for which in range(2):  # 0=q, 1=k
                    base = which * KT
                    cosw = coswk if which else coswq
                    prot = protk if which else protq
                    for i in range(KT):
                        nc.gpsimd.tensor_tensor(xsq[:, i, :], qk[:, base + i, :],
                                                qk[:, base + i, :], op=OP.mult)
                    for i in range(KT):
                        hs = pbc.tile([2, N], F32, tag="bc", name="hs")
                        mm(hs, hsum, xsq[:, i, :], start=True, stop=True)
                        sq2 = tp.tile([2, N], F32, tag="sq2", name="sq2")
                        nc.scalar.activation(sq2, hs, AF.Sqrt, scale=1.0 / HD,
                                             bias=epsc[0:2, 0:1])
                        rs2 = tp.tile([2, N], MMDT, tag="rs2", name="rs2")
                        nc.vector.reciprocal(rs2, sq2)
                        if which == 1:  # rs_k -> transposed into rskT, scaled
                            for t in range(2):
                                tx = pbc.tile([128, 2], MMDT, tag="bc", name="tx")
                                nc.tensor.transpose(tx, rs2[:, t * 128:(t + 1) * 128],
                                                    ident[0:2, 0:2])
                                nc.scalar.activation(rskT[:, t, 2 * i:2 * i + 2], tx,
                                                     AF.Copy, scale=HD ** -0.5)
                        rot = pat.tile([128, N], F32, tag="attn", name="rot")
                        mm(rot, prot, qk[:, base + i, :], start=True, stop=True)
                        m1 = tp.tile([128, N], F32, tag="m1t", name="m1t")
                        nc.vector.tensor_tensor(m1, qk[:, base + i, :], cosw, op=OP.mult)
                        m2t = tp.tile([128, N], F32, tag="m2t", name="m2t")
                        nc.vector.tensor_tensor(m2t, rot, sinm, op=OP.mult)
                        if which == 0:
                            nc.vector.tensor_tensor(m1, m1, m2t, op=OP.add)
                            rbq = pbc.tile([128, N], F32, tag="bc", name="rbq")
                            mm(rbq, sel2, rs2, start=True, stop=True)
                            nc.vector.tensor_tensor(qk[:, base + i, :], m1, rbq, op=OP.mult)
                        else:
                            nc.vector.tensor_tensor(qk[:, base + i, :], m1, m2t, op=OP.add)

                if b == 0:
                    dump("xm0", xm.bitcast(F32))
                    dump("q0", qk[:, 0:KT, :].bitcast(F32))
                    dump("k0", qk[:, KT:2 * KT, :].bitcast(F32))

                # ---- attention (head pairs per feature tile) ----
                for ti in range(KT):
                    rcps = []
                    for hh2 in range(2):
                        h = 2 * ti + hh2
                        po = hh2 * 64
                        expS = tp.tile([128, 2, N], MMDT, tag="expS", name="expS", bufs=2)
                        for kt2 in range(2):
                            st = pmm.tile([128, N], F32, tag="mm", name="st")
                            mm(st, qk[po:po + 64, KT + ti, kt2 * 128:(kt2 + 1) * 128],
                               qk[po:po + 64, ti, :], start=True, stop=True)
                            nc.scalar.activation(expS[:, kt2, :], st, AF.Exp,
                                                 scale=rskT[:, kt2, h:h + 1])
                        oacc = pat.tile([HD + 1, N], F32, tag="attn", name="oacc")
                        for kt2 in range(2):
                            mm(oacc, vaug[:, kt2, h * (HD + 1):(h + 1) * (HD + 1)],
                               expS[:, kt2, :], start=(kt2 == 0), stop=(kt2 == 1))
                        nc.any.tensor_copy(attno[po:po + 64, ti, :], oacc[0:HD, :])
                        rcp = tp.tile([1, N], MMDT, tag="rcph", name="rcph", bufs=3)
                        nc.vector.reciprocal(rcp, oacc[HD:HD + 1, :])
                        rcps.append(rcp)
                    rb2 = pbc.tile([128, N], F32, tag="bc", name="rb2")
                    mm(rb2, halvesrow[0:1, 0:128], rcps[0], start=True, stop=False)
                    mm(rb2, halvesrow[0:1, 128:256], rcps[1], start=False, stop=True)
                    nc.vector.tensor_tensor(attno[:, ti, :], attno[:, ti, :], rb2, op=OP.mult)

                if b == 0:
                    dump("attno0", attno.bitcast(F32))

                # ---- proj + residual ----
                projb = cst.tile([1, C], MMDT, tag="projb", name="projb", bufs=2)
                nc.sync.dma_start(projb, dt["projbr"][b].bitcast(MMDT))
                for ch in range(2):
                    pan = wchunk(dt["wprojT"][b], C, ch * 512, 512, "projch")
                    for j in range(2):
                        co = ch * 2 + j
                        acc = pmm.tile([128, N], F32, tag="mm", name="acc_pj")
                        mm(acc, projb[0:1, co * 128:(co + 1) * 128], onesrow, start=True, stop=False)
                        for k in range(KT):
                            mm(acc, pan[:, k, j * 128:(j + 1) * 128], attno[:, k, :],
                               start=False, stop=(k == KT - 1))
                        nc.vector.scalar_tensor_tensor(xc[:, co, :], acc,
                                                       modscol[:, 16 + co:17 + co], xc[:, co, :],
                                                       op0=OP.mult, op1=OP.add)

                if b == 0:
                    dump("xcp0", xc.bitcast(F32))

                # ---- mlp branch ----
                ln_modulate(xm, True, dt["n2w"][b], a1, modscol, 3, 32)
                for ch in range(6):  # w1/w3 column chunks
                    c0 = ch * 512
                    cwch = min(512, SW - c0)
                    p1 = wchunk(dt["w1T"][b], SW, c0, cwch, "w1ch")
                    p3 = wchunk(dt["w3T"][b], SW, c0, cwch, "w3ch")
                    for j in range((cwch + 127) // 128):
                        co = ch * 4 + j
                        cw = min(128, cwch - j * 128)
                        acc1 = pmm.tile([128, N], F32, tag="mm", name="acc_h1")
                        for k in range(KT):
                            mm(acc1[0:cw, :], p1[:, k, j * 128:j * 128 + cw], xm[:, k, :],
                               start=(k == 0), stop=(k == KT - 1))
                        nc.scalar.activation(h1[0:cw, co, :], acc1[0:cw, :], AF.Silu)
                        acc3 = pmm.tile([128, N], F32, tag="mm", name="acc_h3")
                        for k in range(KT):
                            mm(acc3[0:cw, :], p3[:, k, j * 128:j * 128 + cw], xm[:, k, :],
                               start=(k == 0), stop=(k == KT - 1))
                        nc.vector.tensor_tensor(h1[0:cw, co, :], h1[0:cw, co, :],
                                                acc3[0:cw, :], op=OP.mult)
                for co in range(KT):  # w2 column chunks of 128
                    w2t = w2c.tile([128, SWT, 128], MMDT, tag="w2c", name="w2ch")
                    nc.sync.dma_start(w2t[:, 0:21, :],
                                      dt["w2T"][b, 0:2688, co * 128:(co + 1) * 128]
                                      .rearrange("(a p) n -> p a n", p=128).bitcast(MMDT))
                    nc.sync.dma_start(w2t[0:42, 21, :],
                                      dt["w2T"][b, 2688:2730, co * 128:(co + 1) * 128].bitcast(MMDT))
                    acc = pmm.tile([128, N], F32, tag="mm", name="acc_w2")
                    for k in range(SWT):
                        kp = min(128, SW - k * 128)
                        mm(acc, w2t[0:kp, k, :], h1[0:kp, k, :],
                           start=(k == 0), stop=(k == SWT - 1))
                    nc.vector.scalar_tensor_tensor(xc[:, co, :], acc,
                                                   modscol[:, 40 + co:41 + co], xc[:, co, :],
                                                   op0=OP.mult, op1=OP.add)

                if b == 0:
                    dump("hh0", h1[:, 0:KT, :].bitcast(F32))
                    dump("xc1", xc.bitcast(F32))
            dump("xc4", xc.bitcast(F32))

            # ================= final layer =================
            ada_mods(dt["finadaT"], 2 * C, modsdram)
            nc.sync.dma_start(fincol,
                              modsdram[0:1, 0:2 * C].rearrange("o (j p) -> o p j", p=128)[0])
            nc.vector.tensor_tensor(fincol, fincol, finadabc, op=OP.add)
            ln_modulate(xm, False, None, afin, fincol, 0, 8)
            facc = pmm.tile([16, N], F32, tag="mm", name="facc")
            for k in range(KT):
                mm(facc, finT[:, k, :], xm[:, k, :], start=(k == 0), stop=(k == KT - 1))
            outsb = act.tile([16, N], F32, name="outsb")
            nc.scalar.activation(outsb, facc, AF.Identity, bias=finb[:, 0:1])
            nc.sync.dma_start(out_d[:], outsb)
    ctx_lp.__exit__(None, None, None)
    nc.compile()
    return nc


_NC_CACHE = {}


def host_prep(x, y, cfg_scale, patch_w, patch_b, pos_embed, class_embed,
              cfg_w1, cfg_b1, cfg_w2, cfg_b2,
              blk_norm1_w, blk_norm2_w, blk_qkv_w, blk_proj_w, blk_proj_b,
              blk_qn_w, blk_kn_w, blk_w1, blk_w2, blk_w3, blk_ada_w, blk_ada_b,
              fin_ada_w, fin_ada_b, fin_lin_w, fin_lin_b):
    f = np.float32
    h = 16
    patches = x.reshape(B, 4, h, P, h, P).transpose(0, 2, 4, 1, 3, 5).reshape(B, N, 16)
    hc = cfg_scale[:, None].astype(f) @ cfg_w1.T + cfg_b1
    hc = hc * (1.0 / (1.0 + np.exp(-hc)))
    c = class_embed[y] + hc @ cfg_w2.T + cfg_b2
    silu_c = (c * (1.0 / (1.0 + np.exp(-c)))).astype(f)

    inv = (1.0 / (10000.0 ** (np.arange(0, HD, 2, dtype=np.float64) / HD)))
    fr = np.arange(N, dtype=np.float64)[:, None] * inv[None, :]
    emb = np.concatenate([fr, fr], -1)
    cosT, sinT = np.cos(emb).T.astype(f), np.sin(emb).T.astype(f)  # [64, N]

    def prot_mat(w):  # lhsT for rotate_half with per-d weight folded; 2-head blockdiag
        m = np.zeros((HD, HD), f)
        for d2 in range(32):
            m[d2 + 32, d2] = -w[d2 + 32]   # out[d<32] = -w[d+32]*q[d+32]
            m[d2, d2 + 32] = w[d2]         # out[d>=32] = w[d-32]*q[d-32]
        out = np.zeros((128, 128), f)
        out[:HD, :HD] = m; out[HD:, HD:] = m
        return out

    com = {
        "pos2": np.ascontiguousarray(pos_embed[0].T + patch_b[:, None]).astype(f),
        "patchWT": np.ascontiguousarray(patch_w.T),
        "wqkvT": np.ascontiguousarray(blk_qkv_w.transpose(0, 2, 1)),
        "wprojT": np.ascontiguousarray(blk_proj_w.transpose(0, 2, 1)),
        "projbr": np.ascontiguousarray(blk_proj_b[:, None, :]),
        "w1T": np.ascontiguousarray(blk_w1.transpose(0, 2, 1)),
        "w3T": np.ascontiguousarray(blk_w3.transpose(0, 2, 1)),
        "w2T": np.ascontiguousarray(blk_w2.transpose(0, 2, 1)),
        "adaT": np.ascontiguousarray(blk_ada_w.transpose(0, 2, 1)),
        "adabc": np.ascontiguousarray(blk_ada_b.reshape(D, 48, 128).transpose(0, 2, 1)),
        "n1w": np.ascontiguousarray(blk_norm1_w.reshape(D, KT, 128).transpose(0, 2, 1)),
        "n2w": np.ascontiguousarray(blk_norm2_w.reshape(D, KT, 128).transpose(0, 2, 1)),
        "coswq": np.stack([np.tile(cosT * blk_qn_w[bb][:, None], (2, 1)) for bb in range(D)]),
        "coswk": np.stack([np.tile(cosT * blk_kn_w[bb][:, None], (2, 1)) for bb in range(D)]),
        "sinm": np.tile(sinT, (2, 1)),
        "protq": np.stack([prot_mat(blk_qn_w[bb]) for bb in range(D)]),
        "protk": np.stack([prot_mat(blk_kn_w[bb]) for bb in range(D)]),
        "hsum": np.repeat(np.eye(2, dtype=f), 64, axis=0),
        "sel2": (np.arange(2)[:, None] == np.arange(128)[None, :] // 64).astype(f),
        "halvesrow": np.concatenate([(np.arange(128) < 64).astype(f),
                                     (np.arange(128) >= 64).astype(f)])[None, :],
        "onesrow": np.ones((1, N), f), "onescol": np.ones((128, 1), f),
        "vones": np.ones((128, 32), f),
        "epsc": np.tile(np.array([[EPS_RMS, EPS_LN]], f), (128, 1)),
        "ident": np.eye(128, dtype=f),
        "finadaT": np.ascontiguousarray(fin_ada_w.T),
        "finadabc": np.ascontiguousarray(fin_ada_b.reshape(16, 128).T),
        "finT": np.ascontiguousarray(fin_lin_w.T),
        "finb": np.ascontiguousarray(fin_lin_b[:, None]),
    }
    in_maps = []
    for s in range(B):
        m = dict(com)
        m["patches"] = np.ascontiguousarray(patches[s].T)
        m["siluc"] = np.ascontiguousarray(silu_c[s].reshape(KT, 128).T)
        in_maps.append(m)
    return in_maps


def run(inputs, **kw):
    inputs = {k: np.asarray(v) for k, v in inputs.items()}
    in_maps = host_prep(**inputs)
    if "nc" not in _NC_CACHE:
        _NC_CACHE["nc"] = build_nc()
    nc = _NC_CACHE["nc"]
    res = bass_utils.run_bass_kernel_spmd(nc, in_maps, core_ids=list(range(8)), **kw)
    h = 16
    outs = []
    for s in range(B):
        o = res.results[s]["out"]  # [16, N] = (p1 p2 c, h w)
        full = o.T.reshape(h, h, P, P, OC).transpose(4, 0, 2, 1, 3).reshape(OC, h * P, h * P)
        outs.append(full)
    return np.stack(outs).astype(np.float32), res


def kernel(**inputs):
    out, _ = run(inputs)
    return out


if __name__ == "__main__":
    build_nc()
    print("build ok")



# revision 18
# speedup vs baseline: 1.4641x; 1.4641x over previous
"""DiT forward kernel for Trainium2, data-parallel over batch (1 sample/core x 8 cores).

- Each core runs the full transformer for one batch element.
- Big weights streamed from HBM as float16 (halves DMA vs fp32); activations
  stay float32r (full PE rate at N>=256); PSUM accumulation fp32.
- AdaLN modulation weights sharded 8-way: each core computes its 1/8 column
  slice of silu(c) @ ada^T for ALL samples, then one small AllToAll (106 KB)
  delivers each core its own sample's full modulation vector.
- Partition reductions (rmsnorm/layernorm/softmax-sum) via ones-matmuls on PE;
  partition broadcasts via K=1/selector matmuls; rotate_half via a constant
  permutation matmul with qn/kn weights (and HD^-0.5 for q) folded in.
- q AND k are scaled by their reciprocal-rms via broadcast matmuls (no
  per-token transposes needed).
"""
import sys
sys.path.insert(0, '/opt/trn_rl_repo')
import numpy as np
import concourse.bacc as bacc
import concourse.mybir as mybir
from concourse.tile import TileContext
from concourse import bass_utils

F32 = mybir.dt.float32
F32R = mybir.dt.float32r
F16 = mybir.dt.float16
AF = mybir.ActivationFunctionType
OP = mybir.AluOpType

P, NH, HD, OC, D = 2, 16, 64, 4, 4
B, C, N, SW = 8, 1024, 256, 2730
EPS_RMS, EPS_LN = 1e-6, 1e-5
KT = C // 128             # 8 cin tiles
SWT = (SW + 127) // 128   # 22 sw tiles (last has 42 rows)
GC = 3328                 # ada columns per core (26624 / 8)
NG = GC // 128            # 26 column groups per core
TOTG = 208                # total column groups (4*48 + 16)
GCP = 4096                # GC padded so each A2A shard is a 4096-byte multiple
MMDT = F32R


def build_nc(debug_stage=None):
    nc = bacc.Bacc(None, target_bir_lowering=False, num_devices=8)
    dt = {}

    def din(name, shape, dtype=F32):
        dt[name] = nc.dram_tensor(name, list(shape), dtype, kind="ExternalInput")
        return dt[name]

    din("patches", (16, N), F16); din("pos2", (C, N))
    din("wqkvT", (D, C, 3 * C), F16); din("wprojT", (D, C, C), F16)
    din("projbr", (D, 1, C))
    din("w1T", (D, C, SW), F16); din("w3T", (D, C, SW), F16)
    din("w2T", (D, SW, C), F16)
    din("adachunk", (C, GC), F16); din("siluc16", (C, B), F16)
    din("biascol", (128, TOTG))
    din("n1w", (D, 128, KT)); din("n2w", (D, 128, KT))
    din("coswq", (D, 128, N)); din("coswk", (D, 128, N)); din("sinm", (128, N))
    din("protq", (D, 128, 128), F16); din("protk", (D, 128, 128), F16)
    din("hsum", (128, 2)); din("sel2", (2, 128)); din("halvesrow", (1, 2 * 128))
    din("onesrow", (1, N)); din("onescol", (128, 1)); din("vones", (128, 32), F16)
    din("ident", (128, 128)); din("epsc", (128, 2))
    din("patchWT", (16, C), F16)
    din("finT", (C, 16), F16); din("finb", (16, 1))
    out_d = nc.dram_tensor("out", [16, N], F32, kind="ExternalOutput")
    ccin_d = nc.dram_tensor("ccin", [B, GCP], F32, kind="Internal")
    ccout_d = nc.dram_tensor("ccout", [B, GCP], F32, kind="Internal")

    dbg_tensors = {}

    def dump(stage, ap):
        if not debug_stage:
            return
        shp = [int(s) for s in ap.shape]
        dbg_tensors[stage] = nc.dram_tensor(f"dbg_{stage}", shp, F32, kind="ExternalOutput")
        nc.sync.dma_start(dbg_tensors[stage][:], ap)

    def mm(out, lhsT, rhs, **kw):
        nc.tensor.matmul(out, lhsT, rhs, **kw)

    ctx_lp = nc.allow_low_precision("fp32r/fp16 matmul inputs")
    ctx_lp.__enter__()
    with TileContext(nc) as tc:
        with (
            tc.sbuf_pool(name="cst", bufs=1) as cst,
            tc.sbuf_pool(name="wch", bufs=3) as wch,   # [128, KT, 1024] f16 column chunks
            tc.sbuf_pool(name="w2c", bufs=2) as w2c,   # [128, 22, 256] f16 w2 chunks
            tc.sbuf_pool(name="act", bufs=1) as act,
            tc.sbuf_pool(name="mp", bufs=1) as mp,     # modspart/asb share one slot
            tc.sbuf_pool(name="tp", bufs=2) as tp,
            tc.psum_pool(name="pmm", bufs=4) as pmm,
            tc.psum_pool(name="pat", bufs=2) as pat,
            tc.psum_pool(name="pbc", bufs=2) as pbc,
        ):
            # ---- constants ----
            def cload(name, shape, dtype=F32, **kw):
                t = cst.tile(shape, dtype, name=f"c_{name}", tag=f"c_{name}", **kw)
                src_ap = dt[name][:]
                if dtype not in (F32, F16):
                    src_ap = src_ap.bitcast(dtype)
                nc.sync.dma_start(t, src_ap)
                return t
            patches = cload("patches", [16, N], dtype=F16)
            pos2 = cst.tile([128, KT, N], F32, name="c_pos2", tag="c_pos2")
            nc.sync.dma_start(pos2, dt["pos2"][:].rearrange("(a p) n -> p a n", p=128))
            hsum = cload("hsum", [128, 2], dtype=MMDT)
            sel2 = cload("sel2", [2, 128], dtype=MMDT)
            halvesrow = cload("halvesrow", [1, 2 * 128], dtype=MMDT)
            onesrow = cload("onesrow", [1, N], dtype=MMDT)
            onescol = cload("onescol", [128, 1], dtype=MMDT)
            ident = cload("ident", [128, 128], dtype=MMDT)
            epsc = cload("epsc", [128, 2])
            patchWT = cload("patchWT", [16, C], dtype=F16)
            sinm = cload("sinm", [128, N])
            biascol = cload("biascol", [128, TOTG])
            siluc16 = cst.tile([128, KT, B], F16, name="c_siluc16", tag="c_siluc16")
            nc.sync.dma_start(siluc16, dt["siluc16"][:].rearrange("(a p) s -> p a s", p=128))
            finT16 = cst.tile([128, KT, 16], F16, name="c_finT", tag="c_finT")
            nc.sync.dma_start(finT16, dt["finT"][:].rearrange("(a p) n -> p a n", p=128))
            finb = cload("finb", [16, 1])

            # ---- long-lived activations ----
            xc = act.tile([128, KT, N], MMDT, name="xc")
            xm = act.tile([128, KT, N], F16, name="xm")
            qk = act.tile([128, 2 * KT, N], F16, name="qk")
            vaug = act.tile([128, 2, NH * (HD + 1)], F16, name="vaug")
            attno = act.tile([128, KT, N], F16, name="attno")
            h1 = act.tile([128, SWT, N], F16, name="h1")
            xsq = act.tile([128, KT, N], MMDT, name="xsq")
            modscol = act.tile([128, TOTG], F32, name="modscol")
            modspart = mp.tile([B, GC], F32, tag="mods", name="modspart")
            zpad = mp.tile([B, GCP - GC], F32, tag="zpad", name="zpad")
            asb = mp.tile([B, GC], MMDT, tag="mods", name="asb")
            a1 = act.tile([128, KT], F32, name="a1")
            afin = act.tile([128, KT], F32, name="afin")

            nc.sync.dma_start(
                vaug.rearrange("p t (h d) -> p t h d", d=HD + 1)[:, :, :, HD:HD + 1],
                dt["vones"][:].rearrange("p (t h o) -> p t h o", t=2, h=NH))

            def wchunk(dram_ap, c0, cw, name):
                """Load [C, cw] f16 column chunk as [128, KT, cw]."""
                t = wch.tile([128, KT, cw], F16, tag="wch", name=name,
                             padded_shape=[128, KT, 1024])
                nc.sync.dma_start(t, dram_ap.rearrange("(a p) n -> p a n", p=128)[:, :, c0:c0 + cw])
                return t

            # ================= ada modulations (sharded + AllToAll) =================
            # modspart[s, j] = sum_c silu_c[s, c] * adachunk[c, j]  for this core's slice
            for c0 in (0, 1024, 2048, 3072):
                cw = min(1024, GC - c0)
                pan = wchunk(dt["adachunk"][:], c0, cw, "adach")
                for j0 in range(0, cw, 512):
                    jw = min(512, cw - j0)
                    macc = pmm.tile([B, jw], F32, tag="mm", name="macc",
                                    padded_shape=[128, 512])
                    for k in range(KT):
                        mm(macc, siluc16[:, k, :], pan[:, k, j0:j0 + jw],
                           start=(k == 0), stop=(k == KT - 1))
                    nc.vector.tensor_copy(modspart[:, c0 + j0:c0 + j0 + jw], macc)
            nc.gpsimd.dma_start(ccin_d[:, 0:GC], modspart)
            nc.gpsimd.memset(zpad, 0.0)
            nc.gpsimd.dma_start(ccin_d[:, GC:GCP], zpad)
            nc.gpsimd.collective_compute(
                "AllToAll", OP.bypass,
                replica_groups=[list(range(8))],
                ins=[ccin_d[:].opt()], outs=[ccout_d[:].opt()])
            # read back on gpsimd: same engine queue as the collective trigger, so
            # program order guarantees the collective's completion wait has fired
            nc.gpsimd.dma_start(asb, ccout_d[:, 0:GC].bitcast(MMDT))
            # transpose [8 cores, 128 ch] tiles -> modscol [128, g] with g = r*NG + q
            modsv = modscol.rearrange("p (r q) -> p q r", q=NG)
            biasv = biascol.rearrange("p (r q) -> p q r", q=NG)
            dump("asb", asb.bitcast(F32))
            for q in range(NG):
                tr = pbc.tile([128, B], MMDT, tag="bc", name="tr")
                nc.tensor.transpose(tr, asb[:, q * 128:(q + 1) * 128], ident[0:B, 0:B])
                nc.vector.tensor_tensor(modsv[:, q, :], tr, biasv[:, q, :], op=OP.add)

            dump("mods", modscol)
            # ================= patch embed =================
            for co in range(KT):
                acc = pmm.tile([128, N], F32, tag="mm", name="acc_emb",
                               padded_shape=[128, 512])
                mm(acc, patchWT[:, co * 128:(co + 1) * 128], patches, start=True, stop=True)
                nc.vector.tensor_tensor(xc[:, co, :], acc, pos2[:, co, :], op=OP.add)

            dump("xc0", xc.bitcast(F32))

            def ln_modulate(dst, rms_only, nw_dram, acol, mc, s_shift, s_scale):
                """dst = norm(xc)*(w*(1+scale_mod)) + shift_mod (feature-major)."""
                for i in range(KT):
                    nc.gpsimd.tensor_tensor(xsq[:, i, :], xc[:, i, :], xc[:, i, :], op=OP.mult)
                ssum = pbc.tile([1, N], F32, tag="bc", name="ssum")
                for i in range(KT):
                    mm(ssum, onescol, xsq[:, i, :], start=(i == 0), stop=(i == KT - 1))
                if rms_only:
                    sq = tp.tile([1, N], F32, tag="sq", name="sq", bufs=1)
                    nc.scalar.activation(sq, ssum, AF.Sqrt, scale=1.0 / C, bias=epsc[0:1, 0:1])
                else:
                    s1 = pbc.tile([1, N], F32, tag="bc", name="s1")
                    for i in range(KT):
                        mm(s1, onescol, xc[:, i, :], start=(i == 0), stop=(i == KT - 1))
                    mrow = tp.tile([1, N], F32, tag="mrow", name="mrow", bufs=1)
                    nc.scalar.activation(mrow, s1, AF.Copy, scale=1.0 / C)
                    m2 = tp.tile([1, N], F32, tag="m2", name="m2", bufs=1)
                    nc.scalar.activation(m2, mrow, AF.Square)
                    vrow = tp.tile([1, N], F32, tag="vrow", name="vrow", bufs=1)
                    nc.vector.scalar_tensor_tensor(vrow, ssum, 1.0 / C, m2,
                                                   op0=OP.mult, op1=OP.subtract)
                    sq = tp.tile([1, N], F32, tag="sq", name="sq", bufs=1)
                    nc.scalar.activation(sq, vrow, AF.Sqrt, bias=epsc[0:1, 1:2])
                rsrow = tp.tile([1, N], MMDT, tag="rsrow", name="rsrow", bufs=1)
                nc.vector.reciprocal(rsrow, sq)
                rbc = pbc.tile([128, N], F32, tag="bc", name="rbc")
                mm(rbc, onesrow[:, 0:128], rsrow, start=True, stop=True)
                if not rms_only:
                    mrs = tp.tile([1, N], MMDT, tag="mrs", name="mrs", bufs=1)
                    nc.vector.tensor_tensor(mrs, mrow, rsrow, op=OP.mult)
                    mbc = pbc.tile([128, N], F32, tag="bc", name="mbc")
                    mm(mbc, onesrow[:, 0:128], mrs, start=True, stop=True)
                if nw_dram is None:
                    nc.vector.tensor_scalar_add(acol, mc[:, s_scale:s_scale + KT], 1.0)
                else:
                    aw = cst.tile([128, KT], F32, tag="aw", name="aw", bufs=4)
                    nc.sync.dma_start(aw, nw_dram)
                    nc.vector.scalar_tensor_tensor(acol, mc[:, s_scale:s_scale + KT], 1.0, aw,
                                                   op0=OP.add, op1=OP.mult)
                for i in range(KT):
                    tmp = tp.tile([128, N], F32, tag="lnt", name="lnt")
                    nc.vector.tensor_tensor(tmp, xc[:, i, :], rbc, op=OP.mult)
                    if not rms_only:
                        nc.vector.tensor_tensor(tmp, tmp, mbc, op=OP.subtract)
                    nc.scalar.activation(dst[:, i, :], tmp, AF.Identity,
                                         scale=acol[:, i:i + 1],
                                         bias=mc[:, s_shift * KT + i:s_shift * KT + i + 1])

            # ================= transformer blocks =================
            for b in range(D):
                mc = modscol[:, b * 48:(b + 1) * 48]

                # ---- attn branch ----
                ln_modulate(xm, True, dt["n1w"][b], a1, mc, 0, 8)

                for ch in range(3):  # qkv column chunks of 1024: q, k, v
                    pan = wchunk(dt["wqkvT"][b], ch * 1024, 1024, "qkvch")
                    if ch < 2:  # q, k feature-major
                        for j in range(8):
                            co = ch * 8 + j
                            acc = pmm.tile([128, N], F32, tag="mm", name="acc_qk",
                                           padded_shape=[128, 512])
                            for k in range(KT):
                                mm(acc, pan[:, k, j * 128:(j + 1) * 128], xm[:, k, :],
                                   start=(k == 0), stop=(k == KT - 1))
                            nc.any.tensor_copy(qk[:, co, :], acc)
                    else:  # v token-major into vaug
                        for tq in range(2):
                            for j2 in range(2):
                                acc = pmm.tile([128, 512], F32, tag="mm", name="acc_v")
                                for k in range(KT):
                                    mm(acc, xm[:, k, tq * 128:(tq + 1) * 128],
                                       pan[:, k, j2 * 512:(j2 + 1) * 512],
                                       start=(k == 0), stop=(k == KT - 1))
                                h0 = j2 * 8
                                nc.any.tensor_copy(
                                    vaug[:, tq, :].rearrange("p (h d) -> p h d", d=HD + 1)
                                    [:, h0:h0 + 8, 0:HD],
                                    acc.rearrange("p (h d) -> p h d", d=HD))

                # ---- q/k rmsnorm + rope (rms scale broadcast onto both q and k) ----
                coswq = cst.tile([128, N], F32, tag="coswq", name="coswq", bufs=2)
                nc.sync.dma_start(coswq, dt["coswq"][b])
                coswk = cst.tile([128, N], F32, tag="coswk", name="coswk", bufs=2)
                nc.sync.dma_start(coswk, dt["coswk"][b])
                protq = cst.tile([128, 128], F16, tag="protq", name="protq", bufs=2)
                nc.sync.dma_start(protq, dt["protq"][b])
                protk = cst.tile([128, 128], F16, tag="protk", name="protk", bufs=2)
                nc.sync.dma_start(protk, dt["protk"][b])

                for which in range(2):  # 0=q, 1=k
                    base = which * KT
                    cosw = coswk if which else coswq
                    prot = protk if which else protq
                    for i in range(KT):
                        nc.gpsimd.tensor_tensor(xsq[:, i, :], qk[:, base + i, :],
                                                qk[:, base + i, :], op=OP.mult)
                    for i in range(KT):
                        hs = pbc.tile([2, N], F32, tag="bc", name="hs")
                        mm(hs, hsum, xsq[:, i, :], start=True, stop=True)
                        sq2 = tp.tile([2, N], F32, tag="sq2", name="sq2")
                        nc.scalar.activation(sq2, hs, AF.Sqrt, scale=1.0 / HD,
                                             bias=epsc[0:2, 0:1])
                        rs2 = tp.tile([2, N], MMDT, tag="rs2", name="rs2")
                        nc.vector.reciprocal(rs2, sq2)
                        rot = pat.tile([128, N], F32, tag="attn", name="rot")
                        mm(rot, prot, qk[:, base + i, :], start=True, stop=True)
                        m1 = tp.tile([128, N], F32, tag="m1t", name="m1t")
                        nc.vector.tensor_tensor(m1, qk[:, base + i, :], cosw, op=OP.mult)
                        m2t = tp.tile([128, N], F32, tag="m2t", name="m2t")
                        nc.vector.tensor_tensor(m2t, rot, sinm, op=OP.mult)
                        nc.vector.tensor_tensor(m1, m1, m2t, op=OP.add)
                        rb = pbc.tile([128, N], F32, tag="bc", name="rb")
                        mm(rb, sel2, rs2, start=True, stop=True)
                        nc.vector.tensor_tensor(qk[:, base + i, :], m1, rb, op=OP.mult)

                if b == 0 and debug_stage:
                    for nm, src_ap in (("xm0", xm[:]), ("q0", qk[:, 0:KT, :]), ("k0", qk[:, KT:2 * KT, :])):
                        scr = tp.tile([128, KT, N], F32, tag="dbgs", name=f"dbgs_{nm}", bufs=1)
                        nc.vector.tensor_copy(scr, src_ap)
                        dump(nm, scr)

                # ---- attention (head pairs per feature tile) ----
                for ti in range(KT):
                    rcps = []
                    for hh2 in range(2):
                        h = 2 * ti + hh2
                        po = hh2 * 64
                        expS = tp.tile([128, 2, N], F16, tag="expS", name="expS", bufs=2)
                        for kt2 in range(2):
                            st = pmm.tile([128, N], F32, tag="mm", name="st",
                                          padded_shape=[128, 512])
                            mm(st, qk[po:po + 64, KT + ti, kt2 * 128:(kt2 + 1) * 128],
                               qk[po:po + 64, ti, :], start=True, stop=True)
                            nc.scalar.activation(expS[:, kt2, :], st, AF.Exp)
                        oacc = pat.tile([HD + 1, N], F32, tag="attn", name="oacc")
                        for kt2 in range(2):
                            mm(oacc, vaug[:, kt2, h * (HD + 1):(h + 1) * (HD + 1)],
                               expS[:, kt2, :], start=(kt2 == 0), stop=(kt2 == 1))
                        nc.any.tensor_copy(attno[po:po + 64, ti, :], oacc[0:HD, :])
                        rcp = tp.tile([1, N], MMDT, tag="rcph", name="rcph", bufs=3)
                        nc.vector.reciprocal(rcp, oacc[HD:HD + 1, :])
                        rcps.append(rcp)
                    rb2 = pbc.tile([128, N], F32, tag="bc", name="rb2")
                    mm(rb2, halvesrow[0:1, 0:128], rcps[0], start=True, stop=False)
                    mm(rb2, halvesrow[0:1, 128:256], rcps[1], start=False, stop=True)
                    nc.vector.tensor_tensor(attno[:, ti, :], attno[:, ti, :], rb2, op=OP.mult)

                if b == 0 and debug_stage:
                    scr2 = tp.tile([128, KT, N], F32, tag="dbgs", name="dbgs_attno", bufs=1)
                    nc.vector.tensor_copy(scr2, attno)
                    dump("attno0", scr2)

                # ---- proj + residual ----
                projb = cst.tile([1, C], MMDT, tag="projb", name="projb", bufs=2)
                nc.sync.dma_start(projb, dt["projbr"][b].bitcast(MMDT))
                pan = wchunk(dt["wprojT"][b], 0, 1024, "projch")
                for co in range(KT):
                    acc = pmm.tile([128, N], F32, tag="mm", name="acc_pj",
                                   padded_shape=[128, 512])
                    mm(acc, projb[0:1, co * 128:(co + 1) * 128], onesrow, start=True, stop=False)
                    for k in range(KT):
                        mm(acc, pan[:, k, co * 128:(co + 1) * 128], attno[:, k, :],
                           start=False, stop=(k == KT - 1))
                    nc.vector.scalar_tensor_tensor(xc[:, co, :], acc,
                                                   mc[:, 16 + co:17 + co], xc[:, co, :],
                                                   op0=OP.mult, op1=OP.add)

                if b == 0:
                    dump("xcp0", xc.bitcast(F32))

                # ---- mlp branch ----
                ln_modulate(xm, True, dt["n2w"][b], a1, mc, 3, 32)
                for c0 in (0, 1024, 2048):  # w1 pass: h1 = silu(xm @ w1T)
                    cw = min(1024, SW - c0)
                    p1 = wchunk(dt["w1T"][b], c0, cw, "w1ch")
                    for j in range((cw + 127) // 128):
                        co = c0 // 128 + j
                        jw = min(128, cw - j * 128)
                        acc1 = pmm.tile([128, N], F32, tag="mm", name="acc_h1",
                                        padded_shape=[128, 512])
                        for k in range(KT):
                            mm(acc1[0:jw, :], p1[:, k, j * 128:j * 128 + jw], xm[:, k, :],
                               start=(k == 0), stop=(k == KT - 1))
                        nc.scalar.activation(h1[0:jw, co, :], acc1[0:jw, :], AF.Silu)
                for c0 in (0, 1024, 2048):  # w3 pass: h1 *= xm @ w3T
                    cw = min(1024, SW - c0)
                    p3 = wchunk(dt["w3T"][b], c0, cw, "w3ch")
                    for j in range((cw + 127) // 128):
                        co = c0 // 128 + j
                        jw = min(128, cw - j * 128)
                        acc3 = pmm.tile([128, N], F32, tag="mm", name="acc_h3",
                                        padded_shape=[128, 512])
                        for k in range(KT):
                            mm(acc3[0:jw, :], p3[:, k, j * 128:j * 128 + jw], xm[:, k, :],
                               start=(k == 0), stop=(k == KT - 1))
                        nc.vector.tensor_tensor(h1[0:jw, co, :], h1[0:jw, co, :],
                                                acc3[0:jw, :], op=OP.mult)
                for co2 in range(4):  # w2 column chunks of 256
                    w2t = w2c.tile([128, SWT, 256], F16, tag="w2c", name="w2ch")
                    nc.sync.dma_start(w2t[:, 0:21, :],
                                      dt["w2T"][b, 0:2688, co2 * 256:(co2 + 1) * 256]
                                      .rearrange("(a p) n -> p a n", p=128))
                    nc.sync.dma_start(w2t[0:42, 21, :],
                                      dt["w2T"][b, 2688:2730, co2 * 256:(co2 + 1) * 256])
                    for j in range(2):
                        co = co2 * 2 + j
                        acc = pmm.tile([128, N], F32, tag="mm", name="acc_w2",
                                       padded_shape=[128, 512])
                        for k in range(SWT):
                            kp = min(128, SW - k * 128)
                            mm(acc, w2t[0:kp, k, j * 128:(j + 1) * 128], h1[0:kp, k, :],
                               start=(k == 0), stop=(k == SWT - 1))
                        nc.vector.scalar_tensor_tensor(xc[:, co, :], acc,
                                                       mc[:, 40 + co:41 + co], xc[:, co, :],
                                                       op0=OP.mult, op1=OP.add)

                if b == 0 and debug_stage:
                    scr3 = tp.tile([128, KT, N], F32, tag="dbgs", name="dbgs_hh", bufs=1)
                    nc.vector.tensor_copy(scr3, h1[:, 0:KT, :])
                    dump("hh0", scr3)
                    dump("xc1", xc.bitcast(F32))
            dump("xc4", xc.bitcast(F32))

            # ================= final layer =================
            fincol = modscol[:, 192:TOTG]
            ln_modulate(xm, False, None, afin, fincol, 0, 8)
            facc = pmm.tile([16, N], F32, tag="mm", name="facc", padded_shape=[128, 512])
            for k in range(KT):
                mm(facc, finT16[:, k, :], xm[:, k, :], start=(k == 0), stop=(k == KT - 1))
            outsb = act.tile([16, N], F32, name="outsb")
            nc.scalar.activation(outsb, facc, AF.Identity, bias=finb[:, 0:1])
            nc.sync.dma_start(out_d[:], outsb)
    ctx_lp.__exit__(None, None, None)
    nc.compile()
    return nc


_NC_CACHE = {}


def host_prep(x, y, cfg_scale, patch_w, patch_b, pos_embed, class_embed,
              cfg_w1, cfg_b1, cfg_w2, cfg_b2,
              blk_norm1_w, blk_norm2_w, blk_qkv_w, blk_proj_w, blk_proj_b,
              blk_qn_w, blk_kn_w, blk_w1, blk_w2, blk_w3, blk_ada_w, blk_ada_b,
              fin_ada_w, fin_ada_b, fin_lin_w, fin_lin_b):
    f = np.float32
    h = 16
    patches = x.reshape(B, 4, h, P, h, P).transpose(0, 2, 4, 1, 3, 5).reshape(B, N, 16)
    hc = cfg_scale[:, None].astype(f) @ cfg_w1.T + cfg_b1
    hc = hc * (1.0 / (1.0 + np.exp(-hc)))
    c = class_embed[y] + hc @ cfg_w2.T + cfg_b2
    silu_c = (c * (1.0 / (1.0 + np.exp(-c)))).astype(f)

    inv = (1.0 / (10000.0 ** (np.arange(0, HD, 2, dtype=np.float64) / HD)))
    fr = np.arange(N, dtype=np.float64)[:, None] * inv[None, :]
    emb = np.concatenate([fr, fr], -1)
    cosT, sinT = np.cos(emb).T.astype(f), np.sin(emb).T.astype(f)  # [64, N]
    fq = np.float32(HD ** -0.5)

    def prot_mat(w):  # lhsT for rotate_half with per-d weight folded; 2-head blockdiag
        m = np.zeros((HD, HD), f)
        for d2 in range(32):
            m[d2 + 32, d2] = -w[d2 + 32]   # out[d<32] = -w[d+32]*q[d+32]
            m[d2, d2 + 32] = w[d2]         # out[d>=32] = w[d-32]*q[d-32]
        out = np.zeros((128, 128), f)
        out[:HD, :HD] = m; out[HD:, HD:] = m
        return out

    concatT = np.concatenate(
        [np.ascontiguousarray(blk_ada_w[b].T) for b in range(D)]
        + [np.ascontiguousarray(fin_ada_w.T)], axis=1).astype(f)      # [C, 26624]
    concatb = np.concatenate([blk_ada_b[b] for b in range(D)] + [fin_ada_b])
    biascol = np.ascontiguousarray(concatb.reshape(TOTG, 128).T).astype(f)

    f16 = np.float16
    com = {
        "pos2": np.ascontiguousarray(pos_embed[0].T + patch_b[:, None]).astype(f),
        "patchWT": np.ascontiguousarray(patch_w.T).astype(f16),
        "wqkvT": np.ascontiguousarray(blk_qkv_w.transpose(0, 2, 1)).astype(f16),
        "wprojT": np.ascontiguousarray(blk_proj_w.transpose(0, 2, 1)).astype(f16),
        "projbr": np.ascontiguousarray(blk_proj_b[:, None, :]),
        "w1T": np.ascontiguousarray(blk_w1.transpose(0, 2, 1)).astype(f16),
        "w3T": np.ascontiguousarray(blk_w3.transpose(0, 2, 1)).astype(f16),
        "w2T": np.ascontiguousarray(blk_w2.transpose(0, 2, 1)).astype(f16),
        "siluc16": np.ascontiguousarray(silu_c.T).astype(f16),
        "biascol": biascol,
        "n1w": np.ascontiguousarray(blk_norm1_w.reshape(D, KT, 128).transpose(0, 2, 1)),
        "n2w": np.ascontiguousarray(blk_norm2_w.reshape(D, KT, 128).transpose(0, 2, 1)),
        "coswq": np.stack([np.tile(cosT * blk_qn_w[bb][:, None] * fq, (2, 1)) for bb in range(D)]),
        "coswk": np.stack([np.tile(cosT * blk_kn_w[bb][:, None], (2, 1)) for bb in range(D)]),
        "sinm": np.tile(sinT, (2, 1)),
        "protq": np.stack([prot_mat(blk_qn_w[bb]) * fq for bb in range(D)]).astype(f16),
        "protk": np.stack([prot_mat(blk_kn_w[bb]) for bb in range(D)]).astype(f16),
        "hsum": np.repeat(np.eye(2, dtype=f), 64, axis=0),
        "sel2": (np.arange(2)[:, None] == np.arange(128)[None, :] // 64).astype(f),
        "halvesrow": np.concatenate([(np.arange(128) < 64).astype(f),
                                     (np.arange(128) >= 64).astype(f)])[None, :],
        "onesrow": np.ones((1, N), f), "onescol": np.ones((128, 1), f),
        "vones": np.ones((128, 32), f16),
        "epsc": np.tile(np.array([[EPS_RMS, EPS_LN]], f), (128, 1)),
        "ident": np.eye(128, dtype=f),
        "finT": np.ascontiguousarray(fin_lin_w.T).astype(f16),
        "finb": np.ascontiguousarray(fin_lin_b[:, None]),
    }
    in_maps = []
    for s in range(B):
        m = dict(com)
        m["patches"] = np.ascontiguousarray(patches[s].T).astype(f16)
        m["adachunk"] = np.ascontiguousarray(concatT[:, s * GC:(s + 1) * GC]).astype(f16)
        in_maps.append(m)
    return in_maps


def run(inputs, **kw):
    inputs = {k: np.asarray(v) for k, v in inputs.items()}
    in_maps = host_prep(**inputs)
    if "nc" not in _NC_CACHE:
        _NC_CACHE["nc"] = build_nc()
    nc = _NC_CACHE["nc"]
    res = bass_utils.run_bass_kernel_spmd(nc, in_maps, core_ids=list(range(8)), **kw)
    h = 16
    outs = []
    for s in range(B):
        o = res.results[s]["out"]  # [16, N] = (p1 p2 c, h w)
        full = o.T.reshape(h, h, P, P, OC).transpose(4, 0, 2, 1, 3).reshape(OC, h * P, h * P)
        outs.append(full)
    return np.stack(outs).astype(np.float32), res


def kernel(**inputs):
    out, _ = run(inputs)
    return out


if __name__ == "__main__":
    build_nc()
    print("build ok")


# revision 24
# speedup vs baseline: 1.6012x; 1.0936x over previous
"""DiT forward kernel for Trainium2, data-parallel over batch (1 sample/core x 8 cores).

- Each core runs the full transformer for one batch element.
- Big weights streamed from HBM as float16 (halves DMA vs fp32); activations
  stay float32r (full PE rate at N>=256); PSUM accumulation fp32.
- AdaLN modulation weights sharded 8-way: each core computes its 1/8 column
  slice of silu(c) @ ada^T for ALL samples, then one small AllToAll (106 KB)
  delivers each core its own sample's full modulation vector.
- Partition reductions (rmsnorm/layernorm/softmax-sum) via ones-matmuls on PE;
  partition broadcasts via K=1/selector matmuls; rotate_half via a constant
  permutation matmul with qn/kn weights (and HD^-0.5 for q) folded in.
- q AND k are scaled by their reciprocal-rms via broadcast matmuls (no
  per-token transposes needed).
"""
import sys
sys.path.insert(0, '/opt/trn_rl_repo')
import numpy as np
import concourse.bacc as bacc
import concourse.mybir as mybir
from concourse.tile import TileContext
from concourse import bass_utils

F32 = mybir.dt.float32
F32R = mybir.dt.float32r
F16 = mybir.dt.float16
AF = mybir.ActivationFunctionType
OP = mybir.AluOpType

P, NH, HD, OC, D = 2, 16, 64, 4, 4
B, C, N, SW = 8, 1024, 256, 2730
EPS_RMS, EPS_LN = 1e-6, 1e-5
KT = C // 128             # 8 cin tiles
SWT = (SW + 127) // 128   # 22 sw tiles (last has 42 rows)
GC = 3328                 # ada columns per core (26624 / 8)
NG = GC // 128            # 26 column groups per core
TOTG = 208                # total column groups (4*48 + 16)
GCP = 4096                # GC padded so each A2A shard is a 4096-byte multiple
MMDT = F32R


def build_nc(debug_stage=None):
    nc = bacc.Bacc(None, target_bir_lowering=False, num_devices=8)
    dt = {}

    def din(name, shape, dtype=F32):
        dt[name] = nc.dram_tensor(name, list(shape), dtype, kind="ExternalInput")
        return dt[name]

    din("patches", (16, N), F16); din("pos2", (C, N))
    din("wqkvT", (D, C, 3 * C), F16); din("wprojT", (D, C, C), F16)
    din("projbr", (D, 1, C))
    din("w1T", (D, C, SW), F16); din("w3T", (D, C, SW), F16)
    din("w2T", (D, SW, C), F16)
    din("adachunk", (C, GC), F16); din("siluc16", (C, B), F16)
    din("biascol", (128, TOTG))
    din("n1w", (D, 128, KT)); din("n2w", (D, 128, KT))
    din("coswq", (D, 128, N)); din("coswk", (D, 128, N)); din("sinm", (128, N))
    din("protq", (D, 128, 128), F16); din("protk", (D, 128, 128), F16)
    din("hsum", (128, 2)); din("sel2", (2, 128)); din("halvesrow", (1, 2 * 128))
    din("onesrow", (1, N)); din("onescol", (128, 1)); din("vones", (128, 32), F16)
    din("ident", (128, 128)); din("epsc", (128, 2))
    din("patchWT", (16, C), F16)
    din("finT", (C, 16), F16); din("finb", (16, 1))
    out_d = nc.dram_tensor("out", [16, N], F32, kind="ExternalOutput")
    ccin_d = nc.dram_tensor("ccin", [B, GCP], F32, kind="Internal")
    ccout_d = nc.dram_tensor("ccout", [B, GCP], F32, kind="Internal")

    dbg_tensors = {}

    def dump(stage, ap):
        if not debug_stage:
            return
        shp = [int(s) for s in ap.shape]
        dbg_tensors[stage] = nc.dram_tensor(f"dbg_{stage}", shp, F32, kind="ExternalOutput")
        nc.sync.dma_start(dbg_tensors[stage][:], ap)

    def mm(out, lhsT, rhs, **kw):
        nc.tensor.matmul(out, lhsT, rhs, **kw)

    ctx_lp = nc.allow_low_precision("fp32r/fp16 matmul inputs")
    ctx_lp.__enter__()
    with TileContext(nc) as tc:
        with (
            tc.sbuf_pool(name="cst", bufs=1) as cst,
            tc.sbuf_pool(name="wch", bufs=3) as wch,   # [128, KT, 1024] f16 column chunks
            tc.sbuf_pool(name="w2c", bufs=2) as w2c,   # [128, 22, 256] f16 w2 chunks
            tc.sbuf_pool(name="act", bufs=1) as act,
            tc.sbuf_pool(name="mp", bufs=1) as mp,     # modspart/asb share one slot
            tc.sbuf_pool(name="tp", bufs=2) as tp,
            tc.psum_pool(name="pmm", bufs=4) as pmm,
            tc.psum_pool(name="pat", bufs=2) as pat,
            tc.psum_pool(name="pbc", bufs=2) as pbc,
        ):
            # ---- constants ----
            def cload(name, shape, dtype=F32, **kw):
                t = cst.tile(shape, dtype, name=f"c_{name}", tag=f"c_{name}", **kw)
                src_ap = dt[name][:]
                if dtype not in (F32, F16):
                    src_ap = src_ap.bitcast(dtype)
                nc.sync.dma_start(t, src_ap)
                return t
            patches = cload("patches", [16, N], dtype=F16)
            pos2 = cst.tile([128, KT, N], F32, name="c_pos2", tag="c_pos2")
            nc.sync.dma_start(pos2, dt["pos2"][:].rearrange("(a p) n -> p a n", p=128))
            hsum = cload("hsum", [128, 2], dtype=MMDT)
            sel2 = cload("sel2", [2, 128], dtype=MMDT)
            halvesrow = cload("halvesrow", [1, 2 * 128], dtype=MMDT)
            onesrow = cload("onesrow", [1, N], dtype=MMDT)
            onescol = cload("onescol", [128, 1], dtype=MMDT)
            ident = cload("ident", [128, 128], dtype=MMDT)
            epsc = cload("epsc", [128, 2])
            patchWT = cload("patchWT", [16, C], dtype=F16)
            sinm = cload("sinm", [128, N])
            biascol = cload("biascol", [128, TOTG])
            siluc16 = cst.tile([128, KT, B], F16, name="c_siluc16", tag="c_siluc16")
            nc.sync.dma_start(siluc16, dt["siluc16"][:].rearrange("(a p) s -> p a s", p=128))
            finT16 = cst.tile([128, KT, 16], F16, name="c_finT", tag="c_finT")
            nc.sync.dma_start(finT16, dt["finT"][:].rearrange("(a p) n -> p a n", p=128))
            finb = cload("finb", [16, 1])

            # ---- long-lived activations ----
            xc = act.tile([128, KT, N], MMDT, name="xc")
            xm = act.tile([128, KT, N], F16, name="xm")
            qk = act.tile([128, 2 * KT, N], F16, name="qk")
            vaug = act.tile([128, 2, NH * (HD + 1)], F16, name="vaug")
            attno = act.tile([128, KT, N], F16, name="attno")
            h1 = act.tile([128, SWT, N], F16, name="h1")
            xsq = act.tile([128, KT, N], MMDT, name="xsq")
            modscol = act.tile([128, TOTG], F32, name="modscol")
            modspart = mp.tile([B, GC], F32, tag="mods", name="modspart")
            zpad = mp.tile([B, GCP - GC], F32, tag="zpad", name="zpad")
            asb = mp.tile([B, GC], MMDT, tag="mods", name="asb")
            a1 = act.tile([128, KT], F32, name="a1")
            afin = act.tile([128, KT], F32, name="afin")

            nc.sync.dma_start(
                vaug.rearrange("p t (h d) -> p t h d", d=HD + 1)[:, :, :, HD:HD + 1],
                dt["vones"][:].rearrange("p (t h o) -> p t h o", t=2, h=NH))

            def wchunk(dram_ap, c0, cw, name):
                """Load [C, cw] f16 column chunk as [128, KT, cw]."""
                t = wch.tile([128, KT, cw], F16, tag="wch", name=name,
                             padded_shape=[128, KT, 1024])
                nc.sync.dma_start(t, dram_ap.rearrange("(a p) n -> p a n", p=128)[:, :, c0:c0 + cw])
                return t

            # ================= ada modulations (sharded + AllToAll) =================
            # modspart[s, j] = sum_c silu_c[s, c] * adachunk[c, j]  for this core's slice
            for c0 in (0, 1024, 2048, 3072):
                cw = min(1024, GC - c0)
                pan = wchunk(dt["adachunk"][:], c0, cw, "adach")
                for j0 in range(0, cw, 512):
                    jw = min(512, cw - j0)
                    macc = pmm.tile([B, jw], F32, tag="mm", name="macc",
                                    padded_shape=[128, 512])
                    for k in range(KT):
                        mm(macc, siluc16[:, k, :], pan[:, k, j0:j0 + jw],
                           start=(k == 0), stop=(k == KT - 1))
                    nc.vector.tensor_copy(modspart[:, c0 + j0:c0 + j0 + jw], macc)
            nc.gpsimd.dma_start(ccin_d[:, 0:GC], modspart)
            nc.gpsimd.memset(zpad, 0.0)
            nc.gpsimd.dma_start(ccin_d[:, GC:GCP], zpad)
            nc.gpsimd.collective_compute(
                "AllToAll", OP.bypass,
                replica_groups=[list(range(8))],
                ins=[ccin_d[:].opt()], outs=[ccout_d[:].opt()])
            # read back on gpsimd: same engine queue as the collective trigger, so
            # program order guarantees the collective's completion wait has fired
            nc.gpsimd.dma_start(asb, ccout_d[:, 0:GC].bitcast(MMDT))
            # transpose [8 cores, 128 ch] tiles -> modscol [128, g] with g = r*NG + q
            modsv = modscol.rearrange("p (r q) -> p q r", q=NG)
            biasv = biascol.rearrange("p (r q) -> p q r", q=NG)
            dump("asb", asb.bitcast(F32))
            for q in range(NG):
                tr = pbc.tile([128, B], MMDT, tag="bc", name="tr")
                nc.tensor.transpose(tr, asb[:, q * 128:(q + 1) * 128], ident[0:B, 0:B])
                nc.vector.tensor_tensor(modsv[:, q, :], tr, biasv[:, q, :], op=OP.add)

            dump("mods", modscol)
            # ================= patch embed =================
            for co in range(KT):
                acc = pmm.tile([128, N], F32, tag="mm", name="acc_emb",
                               padded_shape=[128, 512])
                mm(acc, patchWT[:, co * 128:(co + 1) * 128], patches, start=True, stop=True)
                nc.vector.tensor_tensor(xc[:, co, :], acc, pos2[:, co, :], op=OP.add)

            dump("xc0", xc.bitcast(F32))

            def ln_modulate(dst, rms_only, nw_dram, acol, mc, s_shift, s_scale):
                """dst = norm(xc)*(w*(1+scale_mod)) + shift_mod (feature-major)."""
                nc.gpsimd.tensor_tensor(xsq[:], xc[:], xc[:], op=OP.mult)
                ssum = pbc.tile([1, N], F32, tag="bc", name="ssum")
                for i in range(KT):
                    mm(ssum, onescol, xsq[:, i, :], start=(i == 0), stop=(i == KT - 1))
                rsrow = tp.tile([1, N], MMDT, tag="rsrow", name="rsrow", bufs=1)
                if rms_only:
                    nc.scalar.activation(rsrow, ssum, AF.Abs_reciprocal_sqrt, scale=1.0 / C, bias=epsc[0:1, 0:1])
                else:
                    s1 = pbc.tile([1, N], F32, tag="bc", name="s1")
                    for i in range(KT):
                        mm(s1, onescol, xc[:, i, :], start=(i == 0), stop=(i == KT - 1))
                    mrow = tp.tile([1, N], F32, tag="mrow", name="mrow", bufs=1)
                    nc.scalar.activation(mrow, s1, AF.Copy, scale=1.0 / C)
                    m2 = tp.tile([1, N], F32, tag="m2", name="m2", bufs=1)
                    nc.scalar.activation(m2, mrow, AF.Square)
                    vrow = tp.tile([1, N], F32, tag="vrow", name="vrow", bufs=1)
                    nc.vector.scalar_tensor_tensor(vrow, ssum, 1.0 / C, m2,
                                                   op0=OP.mult, op1=OP.subtract)
                    nc.scalar.activation(rsrow, vrow, AF.Abs_reciprocal_sqrt, bias=epsc[0:1, 1:2])
                rbc = pbc.tile([128, N], F32, tag="bc", name="rbc")
                mm(rbc, onesrow[:, 0:128], rsrow, start=True, stop=True)
                if not rms_only:
                    mrs = tp.tile([1, N], MMDT, tag="mrs", name="mrs", bufs=1)
                    nc.vector.tensor_tensor(mrs, mrow, rsrow, op=OP.mult)
                    mbc = pbc.tile([128, N], F32, tag="bc", name="mbc")
                    mm(mbc, onesrow[:, 0:128], mrs, start=True, stop=True)
                if nw_dram is None:
                    nc.vector.tensor_scalar_add(acol, mc[:, s_scale:s_scale + KT], 1.0)
                else:
                    aw = cst.tile([128, KT], F32, tag="aw", name="aw", bufs=4)
                    nc.sync.dma_start(aw, nw_dram)
                    nc.vector.scalar_tensor_tensor(acol, mc[:, s_scale:s_scale + KT], 1.0, aw,
                                                   op0=OP.add, op1=OP.mult)
                for i in range(KT):
                    tmp = tp.tile([128, N], F32, tag="lnt", name="lnt")
                    nc.vector.tensor_tensor(tmp, xc[:, i, :], rbc, op=OP.mult)
                    if not rms_only:
                        nc.vector.tensor_tensor(tmp, tmp, mbc, op=OP.subtract)
                    nc.vector.tensor_scalar(dst[:, i, :], tmp, acol[:, i:i + 1],
                                            mc[:, s_shift * KT + i:s_shift * KT + i + 1],
                                            op0=OP.mult, op1=OP.add)

            # ================= transformer blocks =================
            for b in range(D):
                mc = modscol[:, b * 48:(b + 1) * 48]

                # ---- attn branch ----
                ln_modulate(xm, True, dt["n1w"][b], a1, mc, 0, 8)

                for ch in range(3):  # qkv column chunks of 1024: q, k, v
                    pan = wchunk(dt["wqkvT"][b], ch * 1024, 1024, "qkvch")
                    if ch < 2:  # q, k feature-major
                        for j in range(8):
                            co = ch * 8 + j
                            acc = pmm.tile([128, N], F32, tag="mm", name="acc_qk",
                                           padded_shape=[128, 512])
                            for k in range(KT):
                                mm(acc, pan[:, k, j * 128:(j + 1) * 128], xm[:, k, :],
                                   start=(k == 0), stop=(k == KT - 1))
                            nc.any.tensor_copy(qk[:, co, :], acc)
                    else:  # v token-major into vaug
                        for tq in range(2):
                            for j2 in range(2):
                                acc = pmm.tile([128, 512], F32, tag="mm", name="acc_v")
                                for k in range(KT):
                                    mm(acc, xm[:, k, tq * 128:(tq + 1) * 128],
                                       pan[:, k, j2 * 512:(j2 + 1) * 512],
                                       start=(k == 0), stop=(k == KT - 1))
                                h0 = j2 * 8
                                nc.any.tensor_copy(
                                    vaug[:, tq, :].rearrange("p (h d) -> p h d", d=HD + 1)
                                    [:, h0:h0 + 8, 0:HD],
                                    acc.rearrange("p (h d) -> p h d", d=HD))

                # ---- q/k rmsnorm + rope (rms scale broadcast onto both q and k) ----
                coswq = cst.tile([128, N], F32, tag="coswq", name="coswq", bufs=2)
                nc.sync.dma_start(coswq, dt["coswq"][b])
                coswk = cst.tile([128, N], F32, tag="coswk", name="coswk", bufs=2)
                nc.sync.dma_start(coswk, dt["coswk"][b])
                protq = cst.tile([128, 128], F16, tag="protq", name="protq", bufs=2)
                nc.sync.dma_start(protq, dt["protq"][b])
                protk = cst.tile([128, 128], F16, tag="protk", name="protk", bufs=2)
                nc.sync.dma_start(protk, dt["protk"][b])

                for which in range(2):  # 0=q, 1=k
                    base = which * KT
                    cosw = coswk if which else coswq
                    prot = protk if which else protq
                    nc.gpsimd.tensor_tensor(xsq[:], qk[:, base:base + KT, :],
                                            qk[:, base:base + KT, :], op=OP.mult)
                    for i in range(KT):
                        hs = pbc.tile([2, N], F32, tag="bc", name="hs")
                        mm(hs, hsum, xsq[:, i, :], start=True, stop=True)
                        rs2 = tp.tile([2, N], MMDT, tag="rs2", name="rs2")
                        nc.scalar.activation(rs2, hs, AF.Abs_reciprocal_sqrt, scale=1.0 / HD,
                                             bias=epsc[0:2, 0:1])
                        rot = pat.tile([128, N], F32, tag="attn", name="rot")
                        mm(rot, prot, qk[:, base + i, :], start=True, stop=True)
                        m1 = tp.tile([128, N], F32, tag="m1t", name="m1t")
                        nc.vector.tensor_tensor(m1, qk[:, base + i, :], cosw, op=OP.mult)
                        m2t = tp.tile([128, N], F32, tag="m2t", name="m2t")
                        nc.vector.tensor_tensor(m2t, rot, sinm, op=OP.mult)
                        nc.vector.tensor_tensor(m1, m1, m2t, op=OP.add)
                        rb = pbc.tile([128, N], F32, tag="bc", name="rb")
                        mm(rb, sel2, rs2, start=True, stop=True)
                        nc.vector.tensor_tensor(qk[:, base + i, :], m1, rb, op=OP.mult)

                if b == 0 and debug_stage:
                    for nm, src_ap in (("xm0", xm[:]), ("q0", qk[:, 0:KT, :]), ("k0", qk[:, KT:2 * KT, :])):
                        scr = tp.tile([128, KT, N], F32, tag="dbgs", name=f"dbgs_{nm}", bufs=1)
                        nc.vector.tensor_copy(scr, src_ap)
                        dump(nm, scr)

                # ---- attention (head pairs per feature tile) ----
                for ti in range(KT):
                    rcps = []
                    for hh2 in range(2):
                        h = 2 * ti + hh2
                        po = hh2 * 64
                        expS = tp.tile([128, 2, N], F16, tag="expS", name="expS", bufs=2)
                        st = pmm.tile([128, 2, N], F32, tag="mm", name="st",
                                      padded_shape=[128, 2, 256])
                        for kt2 in range(2):
                            mm(st[:, kt2, :], qk[po:po + 64, KT + ti, kt2 * 128:(kt2 + 1) * 128],
                               qk[po:po + 64, ti, :], start=True, stop=True)
                        nc.scalar.activation(expS, st, AF.Exp)
                        oacc = pat.tile([HD + 1, N], F32, tag="attn", name="oacc")
                        for kt2 in range(2):
                            mm(oacc, vaug[:, kt2, h * (HD + 1):(h + 1) * (HD + 1)],
                               expS[:, kt2, :], start=(kt2 == 0), stop=(kt2 == 1))
                        nc.any.tensor_copy(attno[po:po + 64, ti, :], oacc[0:HD, :])
                        rsq = tp.tile([1, N], F32, tag="rcpf", name="rcpf", bufs=2)
                        nc.scalar.activation(rsq, oacc[HD:HD + 1, :], AF.Abs_reciprocal_sqrt)
                        rcp = tp.tile([1, N], MMDT, tag="rcph", name="rcph", bufs=3)
                        nc.vector.tensor_tensor(rcp, rsq, rsq, op=OP.mult)
                        rcps.append(rcp)
                    rb2 = pbc.tile([128, N], F32, tag="bc", name="rb2")
                    mm(rb2, halvesrow[0:1, 0:128], rcps[0], start=True, stop=False)
                    mm(rb2, halvesrow[0:1, 128:256], rcps[1], start=False, stop=True)
                    nc.vector.tensor_tensor(attno[:, ti, :], attno[:, ti, :], rb2, op=OP.mult)

                if b == 0 and debug_stage:
                    scr2 = tp.tile([128, KT, N], F32, tag="dbgs", name="dbgs_attno", bufs=1)
                    nc.vector.tensor_copy(scr2, attno)
                    dump("attno0", scr2)

                # ---- proj + residual ----
                projb = cst.tile([1, C], MMDT, tag="projb", name="projb", bufs=2)
                nc.sync.dma_start(projb, dt["projbr"][b].bitcast(MMDT))
                pan = wchunk(dt["wprojT"][b], 0, 1024, "projch")
                for co in range(KT):
                    acc = pmm.tile([128, N], F32, tag="mm", name="acc_pj",
                                   padded_shape=[128, 512])
                    mm(acc, projb[0:1, co * 128:(co + 1) * 128], onesrow, start=True, stop=False)
                    for k in range(KT):
                        mm(acc, pan[:, k, co * 128:(co + 1) * 128], attno[:, k, :],
                           start=False, stop=(k == KT - 1))
                    nc.vector.scalar_tensor_tensor(xc[:, co, :], acc,
                                                   mc[:, 16 + co:17 + co], xc[:, co, :],
                                                   op0=OP.mult, op1=OP.add)

                if b == 0:
                    dump("xcp0", xc.bitcast(F32))

                # ---- mlp branch ----
                ln_modulate(xm, True, dt["n2w"][b], a1, mc, 3, 32)
                for c0 in (0, 1024, 2048):  # w1 pass: h1 = silu(xm @ w1T)
                    cw = min(1024, SW - c0)
                    p1 = wchunk(dt["w1T"][b], c0, cw, "w1ch")
                    for j in range((cw + 127) // 128):
                        co = c0 // 128 + j
                        jw = min(128, cw - j * 128)
                        acc1 = pmm.tile([128, N], F32, tag="mm", name="acc_h1",
                                        padded_shape=[128, 512])
                        for k in range(KT):
                            mm(acc1[0:jw, :], p1[:, k, j * 128:j * 128 + jw], xm[:, k, :],
                               start=(k == 0), stop=(k == KT - 1))
                        nc.scalar.activation(h1[0:jw, co, :], acc1[0:jw, :], AF.Silu)
                for c0 in (0, 1024, 2048):  # w3 pass: h1 *= xm @ w3T
                    cw = min(1024, SW - c0)
                    p3 = wchunk(dt["w3T"][b], c0, cw, "w3ch")
                    for j in range((cw + 127) // 128):
                        co = c0 // 128 + j
                        jw = min(128, cw - j * 128)
                        acc3 = pmm.tile([128, N], F32, tag="mm", name="acc_h3",
                                        padded_shape=[128, 512])
                        for k in range(KT):
                            mm(acc3[0:jw, :], p3[:, k, j * 128:j * 128 + jw], xm[:, k, :],
                               start=(k == 0), stop=(k == KT - 1))
                        nc.vector.tensor_tensor(h1[0:jw, co, :], h1[0:jw, co, :],
                                                acc3[0:jw, :], op=OP.mult)
                for co2 in range(4):  # w2 column chunks of 256
                    w2t = w2c.tile([128, SWT, 256], F16, tag="w2c", name="w2ch")
                    nc.sync.dma_start(w2t[:, 0:21, :],
                                      dt["w2T"][b, 0:2688, co2 * 256:(co2 + 1) * 256]
                                      .rearrange("(a p) n -> p a n", p=128))
                    nc.sync.dma_start(w2t[0:42, 21, :],
                                      dt["w2T"][b, 2688:2730, co2 * 256:(co2 + 1) * 256])
                    for j in range(2):
                        co = co2 * 2 + j
                        acc = pmm.tile([128, N], F32, tag="mm", name="acc_w2",
                                       padded_shape=[128, 512])
                        for k in range(SWT):
                            kp = min(128, SW - k * 128)
                            mm(acc, w2t[0:kp, k, j * 128:(j + 1) * 128], h1[0:kp, k, :],
                               start=(k == 0), stop=(k == SWT - 1))
                        nc.vector.scalar_tensor_tensor(xc[:, co, :], acc,
                                                       mc[:, 40 + co:41 + co], xc[:, co, :],
                                                       op0=OP.mult, op1=OP.add)

                if b == 0 and debug_stage:
                    scr3 = tp.tile([128, KT, N], F32, tag="dbgs", name="dbgs_hh", bufs=1)
                    nc.vector.tensor_copy(scr3, h1[:, 0:KT, :])
                    dump("hh0", scr3)
                    dump("xc1", xc.bitcast(F32))
            dump("xc4", xc.bitcast(F32))

            # ================= final layer =================
            fincol = modscol[:, 192:TOTG]
            ln_modulate(xm, False, None, afin, fincol, 0, 8)
            facc = pmm.tile([16, N], F32, tag="mm", name="facc", padded_shape=[128, 512])
            for k in range(KT):
                mm(facc, finT16[:, k, :], xm[:, k, :], start=(k == 0), stop=(k == KT - 1))
            outsb = act.tile([16, N], F32, name="outsb")
            nc.scalar.activation(outsb, facc, AF.Identity, bias=finb[:, 0:1])
            nc.sync.dma_start(out_d[:], outsb)
    ctx_lp.__exit__(None, None, None)
    nc.compile()
    return nc


_NC_CACHE = {}


def host_prep(x, y, cfg_scale, patch_w, patch_b, pos_embed, class_embed,
              cfg_w1, cfg_b1, cfg_w2, cfg_b2,
              blk_norm1_w, blk_norm2_w, blk_qkv_w, blk_proj_w, blk_proj_b,
              blk_qn_w, blk_kn_w, blk_w1, blk_w2, blk_w3, blk_ada_w, blk_ada_b,
              fin_ada_w, fin_ada_b, fin_lin_w, fin_lin_b):
    f = np.float32
    h = 16
    patches = x.reshape(B, 4, h, P, h, P).transpose(0, 2, 4, 1, 3, 5).reshape(B, N, 16)
    hc = cfg_scale[:, None].astype(f) @ cfg_w1.T + cfg_b1
    hc = hc * (1.0 / (1.0 + np.exp(-hc)))
    c = class_embed[y] + hc @ cfg_w2.T + cfg_b2
    silu_c = (c * (1.0 / (1.0 + np.exp(-c)))).astype(f)

    inv = (1.0 / (10000.0 ** (np.arange(0, HD, 2, dtype=np.float64) / HD)))
    fr = np.arange(N, dtype=np.float64)[:, None] * inv[None, :]
    emb = np.concatenate([fr, fr], -1)
    cosT, sinT = np.cos(emb).T.astype(f), np.sin(emb).T.astype(f)  # [64, N]
    fq = np.float32(HD ** -0.5)

    def prot_mat(w):  # lhsT for rotate_half with per-d weight folded; 2-head blockdiag
        m = np.zeros((HD, HD), f)
        for d2 in range(32):
            m[d2 + 32, d2] = -w[d2 + 32]   # out[d<32] = -w[d+32]*q[d+32]
            m[d2, d2 + 32] = w[d2]         # out[d>=32] = w[d-32]*q[d-32]
        out = np.zeros((128, 128), f)
        out[:HD, :HD] = m; out[HD:, HD:] = m
        return out

    concatT = np.concatenate(
        [np.ascontiguousarray(blk_ada_w[b].T) for b in range(D)]
        + [np.ascontiguousarray(fin_ada_w.T)], axis=1).astype(f)      # [C, 26624]
    concatb = np.concatenate([blk_ada_b[b] for b in range(D)] + [fin_ada_b])
    biascol = np.ascontiguousarray(concatb.reshape(TOTG, 128).T).astype(f)

    f16 = np.float16
    com = {
        "pos2": np.ascontiguousarray(pos_embed[0].T + patch_b[:, None]).astype(f),
        "patchWT": np.ascontiguousarray(patch_w.T).astype(f16),
        "wqkvT": np.ascontiguousarray(blk_qkv_w.transpose(0, 2, 1)).astype(f16),
        "wprojT": np.ascontiguousarray(blk_proj_w.transpose(0, 2, 1)).astype(f16),
        "projbr": np.ascontiguousarray(blk_proj_b[:, None, :]),
        "w1T": np.ascontiguousarray(blk_w1.transpose(0, 2, 1)).astype(f16),
        "w3T": np.ascontiguousarray(blk_w3.transpose(0, 2, 1)).astype(f16),
        "w2T": np.ascontiguousarray(blk_w2.transpose(0, 2, 1)).astype(f16),
        "siluc16": np.ascontiguousarray(silu_c.T).astype(f16),
        "biascol": biascol,
        "n1w": np.ascontiguousarray(blk_norm1_w.reshape(D, KT, 128).transpose(0, 2, 1)),
        "n2w": np.ascontiguousarray(blk_norm2_w.reshape(D, KT, 128).transpose(0, 2, 1)),
        "coswq": np.stack([np.tile(cosT * blk_qn_w[bb][:, None] * fq, (2, 1)) for bb in range(D)]),
        "coswk": np.stack([np.tile(cosT * blk_kn_w[bb][:, None], (2, 1)) for bb in range(D)]),
        "sinm": np.tile(sinT, (2, 1)),
        "protq": np.stack([prot_mat(blk_qn_w[bb]) * fq for bb in range(D)]).astype(f16),
        "protk": np.stack([prot_mat(blk_kn_w[bb]) for bb in range(D)]).astype(f16),
        "hsum": np.repeat(np.eye(2, dtype=f), 64, axis=0),
        "sel2": (np.arange(2)[:, None] == np.arange(128)[None, :] // 64).astype(f),
        "halvesrow": np.concatenate([(np.arange(128) < 64).astype(f),
                                     (np.arange(128) >= 64).astype(f)])[None, :],
        "onesrow": np.ones((1, N), f), "onescol": np.ones((128, 1), f),
        "vones": np.ones((128, 32), f16),
        "epsc": np.tile(np.array([[EPS_RMS, EPS_LN]], f), (128, 1)),
        "ident": np.eye(128, dtype=f),
        "finT": np.ascontiguousarray(fin_lin_w.T).astype(f16),
        "finb": np.ascontiguousarray(fin_lin_b[:, None]),
    }
    in_maps = []
    for s in range(B):
        m = dict(com)
        m["patches"] = np.ascontiguousarray(patches[s].T).astype(f16)
        m["adachunk"] = np.ascontiguousarray(concatT[:, s * GC:(s + 1) * GC]).astype(f16)
        in_maps.append(m)
    return in_maps


def run(inputs, **kw):
    inputs = {k: np.asarray(v) for k, v in inputs.items()}
    in_maps = host_prep(**inputs)
    if "nc" not in _NC_CACHE:
        _NC_CACHE["nc"] = build_nc()
    nc = _NC_CACHE["nc"]
    res = bass_utils.run_bass_kernel_spmd(nc, in_maps, core_ids=list(range(8)), **kw)
    h = 16
    outs = []
    for s in range(B):
        o = res.results[s]["out"]  # [16, N] = (p1 p2 c, h w)
        full = o.T.reshape(h, h, P, P, OC).transpose(4, 0, 2, 1, 3).reshape(OC, h * P, h * P)
        outs.append(full)
    return np.stack(outs).astype(np.float32), res


def kernel(**inputs):
    out, _ = run(inputs)
    return out


if __name__ == "__main__":
    build_nc()
    print("build ok")
